# revision 9
# baseline (speedup 1.0000x reference)
"""Bass/Trainium2 kernel for per-chunk fake-quant + linear.

reference semantics (per chunk c):
    q  = clip(round(x/s_c), -128, 127) * s_c
    out[c] = q @ w[c].T          # [B,S,O]

Strategy (v2 - 16-bit I/O, ~32.5MB HBM traffic/core vs 68MB in v1):
  - Data-parallel over tokens: each of 8 cores gets T = B*S/8 = 8192 tokens
    (all 4 chunks), weights replicated.
  - x is staged host-side as f16 [C, D, T] (transposed so the contraction
    dim d sits on SBUF partitions). f16 keeps 11 mantissa bits: the induced
    quant-decision flips add ~0.2% rel error vs the 2e-2 tolerance.
  - Device quantization in two DVE passes (both 16-bit -> 4x perf mode):
      t16 = rne(x * 1/s)   as int16 (HW convert rounds RNE, range +-600)
      qi  = clip(t16, -128, 127) as f16 (exact small integers)
  - Weight-stationary matmuls: lhsT = ws16[c,dk,og] (128d x 128o), rhs =
    qi (128d x N tokens), PSUM tile [128o, 1024t] spanning 2 banks.
    ws16 = (s*w).T * 2^10 as f16 (kept normal), dequant 2^-10 folded into
    the PSUM->SBUF copy.
  - PSUM->SBUF copies at FD=1024 split ACT/DVE; output staged f16 and
    DMA'd out as f16 (host upcasts to f32).
  - DMA rings: x loads on sync HWDGE, weight + output stores on gpsimd
    SWDGE ring.
  - Post-passes: redundant back-to-back LDWEIGHTS with identical source
    APs are replaced by sync-preserving NOPs (weights already resident in
    the PE array); excess per-instruction sem waits hoisted onto NOPs.
"""

import numpy as np

import concourse.bass as bass
import concourse.tile as tile
import concourse.mybir as mybir
from concourse.bass_utils import run_bass_kernel_spmd


def _split_sync_waits(nc):
    """Hoist excess per-instruction sem waits onto preceding same-engine NOPs.

    This walrus build rejects instructions carrying >2 sync waits ("Too many
    sync wait commands", CoreV2/V3GenImpl setupSyncWait). A NOP on the same
    engine immediately before the instruction blocks the queue identically,
    so semantics are preserved.
    """
    count = 0
    for fn in nc.m.functions:
        for bb in fn.blocks:
            out = []
            for ins in bb.instructions:
                si = ins.sync_info
                waits = list(si.on_wait) if (si and si.on_wait) else []
                maxw = 1
                if len(waits) > maxw:
                    extra, keep = waits[:-maxw], waits[-maxw:]
                    ins.sync_info = mybir.SyncInfo(
                        on_wait=keep, on_update=list(si.on_update or [])
                    )
                    for j in range(0, len(extra), maxw):
                        count += 1
                        nop = mybir.InstNoOp(
                            name=f"ant-waitsplit-{count}", ins=[], outs=[]
                        )
                        nop.engine = ins.engine
                        nop.sync_info = mybir.SyncInfo(
                            on_wait=extra[j : j + maxw], on_update=[]
                        )
                        out.append(nop)
                out.append(ins)
            bb.instructions = out
    return count


def _dedupe_ldweights(nc):
    """Replace back-to-back InstLdweights with identical source APs by NOPs.

    The PE array keeps the stationary operand across matmuls; reloading the
    same weights between MMs only costs time. The NOP inherits the LDW's
    sync_info so queue blocking/semaphore semantics are unchanged.
    """
    pe = mybir.EngineType.PE
    count = 0
    for fn in nc.m.functions:
        for bb in fn.blocks:
            last_ldw_key = None
            out = []
            for ins in bb.instructions:
                ty = type(ins).__name__
                if getattr(ins, "engine", None) == pe:
                    if ty == "InstLdweights":
                        key = repr(ins.ins[0])
                        if key == last_ldw_key:
                            count += 1
                            nop = mybir.InstNoOp(
                                name=f"ant-ldwdedupe-{count}", ins=[], outs=[]
                            )
                            nop.engine = ins.engine
                            nop.sync_info = ins.sync_info
                            out.append(nop)
                            continue
                        last_ldw_key = key
                    elif ty in ("InstMatmult", "InstNoOp", "InstEventSemaphore"):
                        pass  # no effect on the PE weight registers
                    else:
                        last_ldw_key = None
                # non-PE instructions don't touch the PE weight registers
                out.append(ins)
            bb.instructions = out
    return count


C, B, S, D, O = 4, 8, 8192, 256, 256
NCORES = 8
N = B * S            # tokens per chunk (65536)
T = N // NCORES      # tokens per chunk per core (8192)

WS_SHIFT = 10           # weights pre-scaled by 2^10 to stay f16-normal
DEQUANT = float(2.0 ** -WS_SHIFT)

TG = 1024               # tokens per tile-group (short pipeline drain tail)
N_TG = T // TG          # 8 tile-groups per chunk per core
SG = 2048               # tokens per output store (2 tile-groups batched)
MM_N = 512              # moving-operand width per matmul (PSUM one-bank limit)
PS_FD = 1024            # PSUM tile free dim (2 banks; copies read both)


def _build_program(scales):
    """Build the SPMD Bass program (same program on all cores).

    Inputs (per core): xt [C, D, T] f16, ws16 [C, D, O] f16.
    Output: out [C, 2, 128, T] f16  (o = og*128 + p, value = true_out).
    """
    f16 = mybir.dt.float16
    f32 = mybir.dt.float32
    i16 = mybir.dt.int16
    alu = mybir.AluOpType

    assert TG == PS_FD and PS_FD % MM_N == 0 and SG == 2 * TG
    n_mm = PS_FD // MM_N       # matmuls per (dk, og) per tile-group

    nc = bass.Bass()
    xt = nc.declare_dram_parameter("xt", [C, D, T], f16, isOutput=False)
    ws16 = nc.declare_dram_parameter("ws16", [C, D, O], f16, isOutput=False)
    out = nc.declare_dram_parameter("out", [C, 2, 128, T], f16, isOutput=True)

    with tile.TileContext(nc) as tc:
        with (
            tc.tile_pool(name="wpool", bufs=1) as wpool,
            tc.tile_pool(name="xpool", bufs=6) as xpool,
            tc.tile_pool(name="tpool", bufs=3) as tpool,
            tc.tile_pool(name="qpool", bufs=4) as qpool,
            tc.tile_pool(name="opool", bufs=4) as opool,
            tc.tile_pool(name="ppool", bufs=4, space=bass.MemorySpace.PSUM) as ppool,
        ):
            # Resident weights: wt[c, dk, og] = [128 d, 128 o] f16.
            w_tile = wpool.tile([128, 2 * C * O], f16, tag="w")
            nc.gpsimd.dma_start(
                out=w_tile[:].rearrange("p (g o) -> p g o", o=O),
                in_=ws16[:].rearrange("c (dk p) o -> p (c dk) o", p=128),
            )
            wt = {}
            for c in range(C):
                for dk in range(2):
                    for og in range(2):
                        base = (c * 2 + dk) * O + og * 128
                        wt[c, dk, og] = w_tile[:, base : base + 128]

            copy_rr = 0  # round-robin over the copy engines
            for c in range(C):
                inv_s = float(np.float32(1.0) / np.float32(scales[c]))
                for tgp in range(N_TG // 2):
                    # Output staging for SG tokens (2 tile-groups), per og.
                    stg = [
                        opool.tile([128, SG], f16, name="stage", tag="stage")
                        for _ in range(2)
                    ]
                    for sub in range(2):
                        tg = tgp * 2 + sub
                        # Load x tile: [p = d%128, (dk, t)]
                        x_tile = xpool.tile([128, 2 * TG], f16, tag="x")
                        src = xt[c].rearrange("(dk p) t -> p dk t", p=128)[
                            :, :, tg * TG : (tg + 1) * TG
                        ]
                        nc.sync.dma_start(
                            out=x_tile[:].rearrange("p (dk t) -> p dk t", dk=2),
                            in_=src,
                        )

                        # t16 = rne(x * inv_s) via HW convert-on-write (RNE).
                        t16 = tpool.tile([128, 2 * TG], i16, tag="t16")
                        nc.vector.tensor_scalar(
                            t16[:], x_tile[:], inv_s, None, alu.mult
                        )
                        # qi = clip(t16, -128, 127) as f16 (exact integers)
                        qi = qpool.tile([128, 2 * TG], f16, tag="qi")
                        nc.vector.tensor_scalar(
                            qi[:], t16[:], -128.0, 127.0, alu.max, alu.min
                        )

                        # Matmuls, weight-stationary: ps [128 o, PS_FD t]
                        for og in range(2):
                            ps = ppool.tile([128, PS_FD], f32, tag="ps")
                            for dk in range(2):
                                for j in range(n_mm):
                                    t0 = dk * TG + j * MM_N
                                    nc.tensor.matmul(
                                        ps[:, j * MM_N : (j + 1) * MM_N],
                                        wt[c, dk, og],
                                        qi[:, t0 : t0 + MM_N],
                                        start=(dk == 0),
                                        stop=(dk == 1),
                                    )
                            # PSUM -> SBUF staging, 2^-10 dequant folded in.
                            dst = stg[og][:, sub * TG : (sub + 1) * TG]
                            # 3 ACT : 1 DVE balances measured engine busy
                            if copy_rr % 4 < 3:
                                nc.scalar.mul(dst, ps[:], DEQUANT)
                            else:
                                nc.vector.tensor_scalar(
                                    dst, ps[:], DEQUANT, None, alu.mult
                                )
                            copy_rr += 1

                    # Store SG tokens per og on the SWDGE ring.
                    for og in range(2):
                        nc.gpsimd.dma_start(
                            out=out[c, og][:, tgp * SG : (tgp + 1) * SG],
                            in_=stg[og][:],
                        )
    return nc


def _prep_inputs(x, w, scales, ncores=NCORES):
    x = np.ascontiguousarray(np.asarray(x, dtype=np.float32)).reshape(C, N, D)
    w = np.asarray(w, dtype=np.float32)
    s = np.asarray(scales, dtype=np.float32).reshape(C, 1, 1)

    ws = s * w                                            # [C, O, D] f32
    wsT = np.ascontiguousarray(ws.transpose(0, 2, 1))     # [C, D, O]
    ws16 = (wsT * np.float32(2.0**WS_SHIFT)).astype(np.float16)

    in_maps = []
    for i in range(ncores):
        xs = x[:, i * T : (i + 1) * T, :]                 # [C, T, D] view
        xtp = np.ascontiguousarray(
            xs.transpose(0, 2, 1).astype(np.float16)      # [C, D, T] f16
        )
        in_maps.append({"xt": xtp, "ws16": ws16})
    return in_maps


def run(x, w, scales, trace=False, **spmd_kwargs):
    """Compile + run on 8 cores. Returns (out, BassKernelResults)."""
    scales = np.asarray(scales, dtype=np.float32)
    nc = _build_program(scales)
    _dedupe_ldweights(nc)
    _split_sync_waits(nc)  # HW-only fixup (CoreSim chokes on raw-BIR NoOps)
    in_maps = _prep_inputs(x, w, scales)
    res = run_bass_kernel_spmd(
        nc, in_maps, core_ids=list(range(NCORES)), trace=trace, **spmd_kwargs
    )
    # Un-permute each shard: [C, 2, 128, T] (c, og, oj, t) -> [C, T, O]
    shards = [
        r["out"].reshape(C, O, T).transpose(0, 2, 1).astype(np.float32)
        for r in res.results
    ]
    out = np.concatenate(shards, axis=1)                  # [C, N, O]
    return np.ascontiguousarray(out).reshape(C, B, S, O), res


def kernel(x, w, scales):
    out, _ = run(x, w, scales, trace=False)
    return out


# revision 13
# speedup vs baseline: 1.1774x; 1.1774x over previous
"""Bass/Trainium2 kernel for per-chunk fake-quant + linear.

reference semantics (per chunk c):
    q  = clip(round(x/s_c), -128, 127) * s_c
    out[c] = q @ w[c].T          # [B,S,O]

Strategy (v2 - 16-bit I/O, ~32.5MB HBM traffic/core vs 68MB in v1):
  - Data-parallel over tokens: each of 8 cores gets T = B*S/8 = 8192 tokens
    (all 4 chunks), weights replicated.
  - x is staged host-side as f16 [C, D, T] (transposed so the contraction
    dim d sits on SBUF partitions). f16 keeps 11 mantissa bits: the induced
    quant-decision flips add ~0.2% rel error vs the 2e-2 tolerance.
  - Device quantization in two DVE passes (both 16-bit -> 4x perf mode):
      t16 = rne(x * 1/s)   as int16 (HW convert rounds RNE, range +-600)
      qi  = clip(t16, -128, 127) as f16 (exact small integers)
  - Weight-stationary matmuls: lhsT = ws16[c,dk,og] (128d x 128o), rhs =
    qi (128d x N tokens), PSUM tile [128o, 1024t] spanning 2 banks.
    ws16 = (s*w).T * 2^10 as f16 (kept normal), dequant 2^-10 folded into
    the PSUM->SBUF copy.
  - PSUM->SBUF copies at FD=1024 split ACT/DVE; output staged f16 and
    DMA'd out as f16 (host upcasts to f32).
  - DMA rings: x loads on sync HWDGE, weight + output stores on gpsimd
    SWDGE ring.
  - Post-passes: redundant back-to-back LDWEIGHTS with identical source
    APs are replaced by sync-preserving NOPs (weights already resident in
    the PE array); excess per-instruction sem waits hoisted onto NOPs.
"""

import numpy as np

import concourse.bass as bass
import concourse.tile as tile
import concourse.mybir as mybir
from concourse.bass_utils import run_bass_kernel_spmd


def _split_sync_waits(nc):
    """Hoist excess per-instruction sem waits onto preceding same-engine NOPs.

    This walrus build rejects instructions carrying >2 sync waits ("Too many
    sync wait commands", CoreV2/V3GenImpl setupSyncWait). A NOP on the same
    engine immediately before the instruction blocks the queue identically,
    so semantics are preserved.
    """
    count = 0
    for fn in nc.m.functions:
        for bb in fn.blocks:
            out = []
            for ins in bb.instructions:
                si = ins.sync_info
                waits = list(si.on_wait) if (si and si.on_wait) else []
                maxw = 1
                if len(waits) > maxw:
                    extra, keep = waits[:-maxw], waits[-maxw:]
                    ins.sync_info = mybir.SyncInfo(
                        on_wait=keep, on_update=list(si.on_update or [])
                    )
                    for j in range(0, len(extra), maxw):
                        count += 1
                        nop = mybir.InstNoOp(
                            name=f"ant-waitsplit-{count}", ins=[], outs=[]
                        )
                        nop.engine = ins.engine
                        nop.sync_info = mybir.SyncInfo(
                            on_wait=extra[j : j + maxw], on_update=[]
                        )
                        out.append(nop)
                out.append(ins)
            bb.instructions = out
    return count


def _dedupe_ldweights(nc):
    """Replace back-to-back InstLdweights with identical source APs by NOPs.

    The PE array keeps the stationary operand across matmuls; reloading the
    same weights between MMs only costs time. The NOP inherits the LDW's
    sync_info so queue blocking/semaphore semantics are unchanged.
    """
    pe = mybir.EngineType.PE
    count = 0
    for fn in nc.m.functions:
        for bb in fn.blocks:
            last_ldw_key = None
            out = []
            for ins in bb.instructions:
                ty = type(ins).__name__
                if getattr(ins, "engine", None) == pe:
                    if ty == "InstLdweights":
                        key = repr(ins.ins[0])
                        if key == last_ldw_key:
                            count += 1
                            nop = mybir.InstNoOp(
                                name=f"ant-ldwdedupe-{count}", ins=[], outs=[]
                            )
                            nop.engine = ins.engine
                            nop.sync_info = ins.sync_info
                            out.append(nop)
                            continue
                        last_ldw_key = key
                    elif ty in ("InstMatmult", "InstNoOp", "InstEventSemaphore"):
                        pass  # no effect on the PE weight registers
                    else:
                        last_ldw_key = None
                # non-PE instructions don't touch the PE weight registers
                out.append(ins)
            bb.instructions = out
    return count


C, B, S, D, O = 4, 8, 8192, 256, 256
NCORES = 8
N = B * S            # tokens per chunk (65536)
T = N // NCORES      # tokens per chunk per core (8192)

WS_SHIFT = 10           # weights pre-scaled by 2^10 to stay f16-normal
DEQUANT = float(2.0 ** -WS_SHIFT)

TG = 2048               # tokens per tile-group
N_TG = T // TG          # 4 tile-groups per chunk per core
MM_N = 512              # moving-operand width per matmul (PSUM one-bank limit)
PS_FD = 1024            # PSUM tile free dim (2 banks; copies read both)


def _build_program(scales):
    """Build the SPMD Bass program (same program on all cores).

    Inputs (per core): xt [C, D, T] f16, ws16 [C, D, O] f16.
    Output: out [C, 2, 128, T] f16  (o = og*128 + p, value = true_out).
    """
    f16 = mybir.dt.float16
    f32 = mybir.dt.float32
    i16 = mybir.dt.int16
    alu = mybir.AluOpType

    assert TG % PS_FD == 0 and PS_FD % MM_N == 0
    n_ps = TG // PS_FD         # PSUM tiles per og per tile-group
    n_mm = PS_FD // MM_N       # matmuls per PSUM tile per dk

    nc = bass.Bass()
    xt = nc.declare_dram_parameter("xt", [C, D, T], f16, isOutput=False)
    ws16 = nc.declare_dram_parameter("ws16", [C, D, O], f16, isOutput=False)
    out = nc.declare_dram_parameter("out", [C, 2, 128, T], f16, isOutput=True)

    with tile.TileContext(nc) as tc:
        with (
            tc.tile_pool(name="wpool", bufs=1) as wpool,
            tc.tile_pool(name="xpool", bufs=6) as xpool,
            tc.tile_pool(name="tpool", bufs=3) as tpool,
            tc.tile_pool(name="qpool", bufs=4) as qpool,
            tc.tile_pool(name="opool", bufs=4) as opool,
            tc.tile_pool(name="ppool", bufs=4, space=bass.MemorySpace.PSUM) as ppool,
        ):
            # Resident weights: wt[c, dk, og] = [128 d, 128 o] f16.
            w_tile = wpool.tile([128, 2 * C * O], f16, tag="w")
            nc.gpsimd.dma_start(
                out=w_tile[:].rearrange("p (g o) -> p g o", o=O),
                in_=ws16[:].rearrange("c (dk p) o -> p (c dk) o", p=128),
            )
            wt = {}
            for c in range(C):
                for dk in range(2):
                    for og in range(2):
                        base = (c * 2 + dk) * O + og * 128
                        wt[c, dk, og] = w_tile[:, base : base + 128]

            copy_rr = 0  # round-robin over the copy engines
            for c in range(C):
                inv_s = float(np.float32(1.0) / np.float32(scales[c]))
                for tg in range(N_TG):
                    # Load x tile: [p = d%128, (dk, t)]
                    x_tile = xpool.tile([128, 2 * TG], f16, tag="x")
                    src = xt[c].rearrange("(dk p) t -> p dk t", p=128)[
                        :, :, tg * TG : (tg + 1) * TG
                    ]
                    nc.sync.dma_start(
                        out=x_tile[:].rearrange("p (dk t) -> p dk t", dk=2),
                        in_=src,
                    )

                    # t16 = rne(x * inv_s) via the HW convert-on-write (RNE).
                    t16 = tpool.tile([128, 2 * TG], i16, tag="t16")
                    nc.vector.tensor_scalar(
                        t16[:], x_tile[:], inv_s, None, alu.mult
                    )
                    # qi = clip(t16, -128, 127) as f16 (exact integers)
                    qi = qpool.tile([128, 2 * TG], f16, tag="qi")
                    nc.vector.tensor_scalar(
                        qi[:], t16[:], -128.0, 127.0, alu.max, alu.min
                    )

                    # Matmuls, weight-stationary: ps [128 o, PS_FD t]
                    stage = opool.tile([128, 2 * TG], f16, tag="stage")
                    for og in range(2):
                        pss = [
                            ppool.tile([128, PS_FD], f32, name="ps", tag="ps")
                            for _ in range(n_ps)
                        ]
                        for dk in range(2):
                            for ips, ps in enumerate(pss):
                                for j in range(n_mm):
                                    t0 = ips * PS_FD + j * MM_N
                                    nc.tensor.matmul(
                                        ps[:, j * MM_N : (j + 1) * MM_N],
                                        wt[c, dk, og],
                                        qi[:, dk * TG + t0 : dk * TG + t0 + MM_N],
                                        start=(dk == 0),
                                        stop=(dk == 1),
                                    )
                        # PSUM -> SBUF staging with 2^-10 dequant folded in.
                        for ips, ps in enumerate(pss):
                            dst = stage[
                                :, og * TG + ips * PS_FD : og * TG + (ips + 1) * PS_FD
                            ]
                            # 3 ACT : 1 DVE balances measured engine busy
                            if copy_rr % 4 < 3:
                                nc.scalar.mul(dst, ps[:], DEQUANT)
                            else:
                                nc.vector.tensor_scalar(
                                    dst, ps[:], DEQUANT, None, alu.mult
                                )
                            copy_rr += 1

                    # Store both og halves in one 1MB SWDGE DMA:
                    # stage [p, (og t)] -> out[c, og, p, tg*TG:(tg+1)*TG]
                    nc.gpsimd.dma_start(
                        out=out[c]
                        .rearrange("og p t -> p og t")[
                            :, :, tg * TG : (tg + 1) * TG
                        ],
                        in_=stage[:].rearrange("p (og t) -> p og t", og=2),
                    )
    return nc


def _prep_inputs(x, w, scales, ncores=NCORES):
    x = np.ascontiguousarray(np.asarray(x, dtype=np.float32)).reshape(C, N, D)
    w = np.asarray(w, dtype=np.float32)
    s = np.asarray(scales, dtype=np.float32).reshape(C, 1, 1)

    ws = s * w                                            # [C, O, D] f32
    wsT = np.ascontiguousarray(ws.transpose(0, 2, 1))     # [C, D, O]
    ws16 = (wsT * np.float32(2.0**WS_SHIFT)).astype(np.float16)

    in_maps = []
    for i in range(ncores):
        xs = x[:, i * T : (i + 1) * T, :]                 # [C, T, D] view
        xtp = np.ascontiguousarray(
            xs.transpose(0, 2, 1).astype(np.float16)      # [C, D, T] f16
        )
        in_maps.append({"xt": xtp, "ws16": ws16})
    return in_maps


def run(x, w, scales, trace=False, **spmd_kwargs):
    """Compile + run on 8 cores. Returns (out, BassKernelResults)."""
    scales = np.asarray(scales, dtype=np.float32)
    nc = _build_program(scales)
    _dedupe_ldweights(nc)
    _split_sync_waits(nc)  # HW-only fixup (CoreSim chokes on raw-BIR NoOps)
    in_maps = _prep_inputs(x, w, scales)
    res = run_bass_kernel_spmd(
        nc, in_maps, core_ids=list(range(NCORES)), trace=trace, **spmd_kwargs
    )
    # Un-permute each shard: [C, 2, 128, T] (c, og, oj, t) -> [C, T, O]
    shards = [
        r["out"].reshape(C, O, T).transpose(0, 2, 1).astype(np.float32)
        for r in res.results
    ]
    out = np.concatenate(shards, axis=1)                  # [C, N, O]
    return np.ascontiguousarray(out).reshape(C, B, S, O), res


def kernel(x, w, scales):
    out, _ = run(x, w, scales, trace=False)
    return out


# revision 15
# speedup vs baseline: 1.1860x; 1.0073x over previous
"""Bass/Trainium2 kernel for per-chunk fake-quant + linear.

reference semantics (per chunk c):
    q  = clip(round(x/s_c), -128, 127) * s_c
    out[c] = q @ w[c].T          # [B,S,O]

Strategy (v2 - 16-bit I/O, ~32.5MB HBM traffic/core vs 68MB in v1):
  - Data-parallel over tokens: each of 8 cores gets T = B*S/8 = 8192 tokens
    (all 4 chunks), weights replicated.
  - x is staged host-side as f16 [C, D, T] (transposed so the contraction
    dim d sits on SBUF partitions). f16 keeps 11 mantissa bits: the induced
    quant-decision flips add ~0.2% rel error vs the 2e-2 tolerance.
  - Device quantization in two DVE passes (both 16-bit -> 4x perf mode):
      t16 = rne(x * 1/s)   as int16 (HW convert rounds RNE, range +-600)
      qi  = clip(t16, -128, 127) as f16 (exact small integers)
  - Weight-stationary matmuls: lhsT = ws16[c,dk,og] (128d x 128o), rhs =
    qi (128d x N tokens), PSUM tile [128o, 1024t] spanning 2 banks.
    ws16 = (s*w).T * 2^10 as f16 (kept normal), dequant 2^-10 folded into
    the PSUM->SBUF copy.
  - PSUM->SBUF copies at FD=1024 split ACT/DVE; output staged f16 and
    DMA'd out as f16 (host upcasts to f32).
  - DMA rings: x loads on sync HWDGE, weight + output stores on gpsimd
    SWDGE ring.
  - Post-passes: redundant back-to-back LDWEIGHTS with identical source
    APs are replaced by sync-preserving NOPs (weights already resident in
    the PE array); excess per-instruction sem waits hoisted onto NOPs.
"""

import numpy as np

import concourse.bass as bass
import concourse.tile as tile
import concourse.mybir as mybir
from concourse.bass_utils import run_bass_kernel_spmd


def _split_sync_waits(nc):
    """Hoist excess per-instruction sem waits onto preceding same-engine NOPs.

    This walrus build rejects instructions carrying >2 sync waits ("Too many
    sync wait commands", CoreV2/V3GenImpl setupSyncWait). A NOP on the same
    engine immediately before the instruction blocks the queue identically,
    so semantics are preserved.
    """
    count = 0
    for fn in nc.m.functions:
        for bb in fn.blocks:
            out = []
            for ins in bb.instructions:
                si = ins.sync_info
                waits = list(si.on_wait) if (si and si.on_wait) else []
                maxw = 1
                if len(waits) > maxw:
                    extra, keep = waits[:-maxw], waits[-maxw:]
                    ins.sync_info = mybir.SyncInfo(
                        on_wait=keep, on_update=list(si.on_update or [])
                    )
                    for j in range(0, len(extra), maxw):
                        count += 1
                        nop = mybir.InstNoOp(
                            name=f"ant-waitsplit-{count}", ins=[], outs=[]
                        )
                        nop.engine = ins.engine
                        nop.sync_info = mybir.SyncInfo(
                            on_wait=extra[j : j + maxw], on_update=[]
                        )
                        out.append(nop)
                out.append(ins)
            bb.instructions = out
    return count


def _dedupe_ldweights(nc):
    """Replace back-to-back InstLdweights with identical source APs by NOPs.

    The PE array keeps the stationary operand across matmuls; reloading the
    same weights between MMs only costs time. The NOP inherits the LDW's
    sync_info so queue blocking/semaphore semantics are unchanged.
    """
    pe = mybir.EngineType.PE
    count = 0
    for fn in nc.m.functions:
        for bb in fn.blocks:
            last_ldw_key = None
            out = []
            for ins in bb.instructions:
                ty = type(ins).__name__
                if getattr(ins, "engine", None) == pe:
                    if ty == "InstLdweights":
                        key = repr(ins.ins[0])
                        if key == last_ldw_key:
                            count += 1
                            nop = mybir.InstNoOp(
                                name=f"ant-ldwdedupe-{count}", ins=[], outs=[]
                            )
                            nop.engine = ins.engine
                            nop.sync_info = ins.sync_info
                            out.append(nop)
                            continue
                        last_ldw_key = key
                    elif ty in ("InstMatmult", "InstNoOp", "InstEventSemaphore"):
                        pass  # no effect on the PE weight registers
                    else:
                        last_ldw_key = None
                # non-PE instructions don't touch the PE weight registers
                out.append(ins)
            bb.instructions = out
    return count


C, B, S, D, O = 4, 8, 8192, 256, 256
NCORES = 8
N = B * S            # tokens per chunk (65536)
T = N // NCORES      # tokens per chunk per core (8192)

WS_SHIFT = 10           # weights pre-scaled by 2^10 to stay f16-normal
DEQUANT = float(2.0 ** -WS_SHIFT)

TG = 2048               # tokens per tile-group
N_TG = T // TG          # 4 tile-groups per chunk per core
MM_N = 512              # moving-operand width per matmul (PSUM one-bank limit)
PS_FD = 1024            # PSUM tile free dim (2 banks; copies read both)


def _build_program(scales):
    """Build the SPMD Bass program (same program on all cores).

    Inputs (per core): xt [C, D, T] f16, ws16 [C, D, O] f16.
    Output: out [C, 2, 128, T] f16  (o = og*128 + p, value = true_out).
    """
    f16 = mybir.dt.float16
    f32 = mybir.dt.float32
    i16 = mybir.dt.int16
    alu = mybir.AluOpType

    assert TG % PS_FD == 0 and PS_FD % MM_N == 0
    n_ps = TG // PS_FD         # PSUM tiles per og per tile-group
    n_mm = PS_FD // MM_N       # matmuls per PSUM tile per dk

    nc = bass.Bass()
    xt = nc.declare_dram_parameter("xt", [C, D, T], f16, isOutput=False)
    ws16 = nc.declare_dram_parameter("ws16", [C, D, O], f16, isOutput=False)
    out = nc.declare_dram_parameter("out", [C, 2, 128, T], f16, isOutput=True)

    with tile.TileContext(nc) as tc:
        with (
            tc.tile_pool(name="wpool", bufs=1) as wpool,
            tc.tile_pool(name="xpool", bufs=8) as xpool,
            tc.tile_pool(name="tpool", bufs=3) as tpool,
            tc.tile_pool(name="qpool", bufs=4) as qpool,
            tc.tile_pool(name="opool", bufs=5) as opool,
            tc.tile_pool(name="ppool", bufs=4, space=bass.MemorySpace.PSUM) as ppool,
        ):
            # Resident weights: wt[c, dk, og] = [128 d, 128 o] f16.
            w_tile = wpool.tile([128, 2 * C * O], f16, tag="w")
            nc.gpsimd.dma_start(
                out=w_tile[:].rearrange("p (g o) -> p g o", o=O),
                in_=ws16[:].rearrange("c (dk p) o -> p (c dk) o", p=128),
            )
            wt = {}
            for c in range(C):
                for dk in range(2):
                    for og in range(2):
                        base = (c * 2 + dk) * O + og * 128
                        wt[c, dk, og] = w_tile[:, base : base + 128]

            copy_rr = 0  # round-robin over the copy engines
            for c in range(C):
                inv_s = float(np.float32(1.0) / np.float32(scales[c]))
                for tg in range(N_TG):
                    # Load x tile: [p = d%128, (dk, t)]
                    x_tile = xpool.tile([128, 2 * TG], f16, tag="x")
                    src = xt[c].rearrange("(dk p) t -> p dk t", p=128)[
                        :, :, tg * TG : (tg + 1) * TG
                    ]
                    nc.sync.dma_start(
                        out=x_tile[:].rearrange("p (dk t) -> p dk t", dk=2),
                        in_=src,
                    )

                    # t16 = rne(x * inv_s) via the HW convert-on-write (RNE).
                    t16 = tpool.tile([128, 2 * TG], i16, tag="t16")
                    nc.vector.tensor_scalar(
                        t16[:], x_tile[:], inv_s, None, alu.mult
                    )
                    # qi = clip(t16, -128, 127) as f16 (exact integers)
                    qi = qpool.tile([128, 2 * TG], f16, tag="qi")
                    nc.vector.tensor_scalar(
                        qi[:], t16[:], -128.0, 127.0, alu.max, alu.min
                    )

                    # Matmuls, weight-stationary: ps [128 o, PS_FD t]
                    stage = opool.tile([128, 2 * TG], f16, tag="stage")
                    for og in range(2):
                        pss = [
                            ppool.tile([128, PS_FD], f32, name="ps", tag="ps")
                            for _ in range(n_ps)
                        ]
                        for dk in range(2):
                            for ips, ps in enumerate(pss):
                                for j in range(n_mm):
                                    t0 = ips * PS_FD + j * MM_N
                                    nc.tensor.matmul(
                                        ps[:, j * MM_N : (j + 1) * MM_N],
                                        wt[c, dk, og],
                                        qi[:, dk * TG + t0 : dk * TG + t0 + MM_N],
                                        start=(dk == 0),
                                        stop=(dk == 1),
                                    )
                        # PSUM -> SBUF staging with 2^-10 dequant folded in.
                        for ips, ps in enumerate(pss):
                            dst = stage[
                                :, og * TG + ips * PS_FD : og * TG + (ips + 1) * PS_FD
                            ]
                            # 3 ACT : 1 DVE balances measured engine busy
                            if copy_rr % 4 < 3:
                                nc.scalar.mul(dst, ps[:], DEQUANT)
                            else:
                                nc.vector.tensor_scalar(
                                    dst, ps[:], DEQUANT, None, alu.mult
                                )
                            copy_rr += 1

                    # Store both og halves in one 1MB SWDGE DMA:
                    # stage [p, (og t)] -> out[c, og, p, tg*TG:(tg+1)*TG]
                    if c < C - 1:
                        nc.gpsimd.dma_start(
                            out=out[c]
                            .rearrange("og p t -> p og t")[
                                :, :, tg * TG : (tg + 1) * TG
                            ],
                            in_=stage[:].rearrange("p (og t) -> p og t", og=2),
                        )
                    else:
                        # Last chunk: drain the tail over the (by now idle)
                        # HWDGE rings in parallel with Pool's earlier stores.
                        for og, eng in ((0, nc.sync), (1, nc.scalar)):
                            eng.dma_start(
                                out=out[c, og][:, tg * TG : (tg + 1) * TG],
                                in_=stage[:, og * TG : (og + 1) * TG],
                            )
    return nc


def _prep_inputs(x, w, scales, ncores=NCORES):
    x = np.ascontiguousarray(np.asarray(x, dtype=np.float32)).reshape(C, N, D)
    w = np.asarray(w, dtype=np.float32)
    s = np.asarray(scales, dtype=np.float32).reshape(C, 1, 1)

    ws = s * w                                            # [C, O, D] f32
    wsT = np.ascontiguousarray(ws.transpose(0, 2, 1))     # [C, D, O]
    ws16 = (wsT * np.float32(2.0**WS_SHIFT)).astype(np.float16)

    in_maps = []
    for i in range(ncores):
        xs = x[:, i * T : (i + 1) * T, :]                 # [C, T, D] view
        xtp = np.ascontiguousarray(
            xs.transpose(0, 2, 1).astype(np.float16)      # [C, D, T] f16
        )
        in_maps.append({"xt": xtp, "ws16": ws16})
    return in_maps


def run(x, w, scales, trace=False, **spmd_kwargs):
    """Compile + run on 8 cores. Returns (out, BassKernelResults)."""
    scales = np.asarray(scales, dtype=np.float32)
    nc = _build_program(scales)
    _dedupe_ldweights(nc)
    _split_sync_waits(nc)  # HW-only fixup (CoreSim chokes on raw-BIR NoOps)
    in_maps = _prep_inputs(x, w, scales)
    res = run_bass_kernel_spmd(
        nc, in_maps, core_ids=list(range(NCORES)), trace=trace, **spmd_kwargs
    )
    # Un-permute each shard: [C, 2, 128, T] (c, og, oj, t) -> [C, T, O]
    shards = [
        r["out"].reshape(C, O, T).transpose(0, 2, 1).astype(np.float32)
        for r in res.results
    ]
    out = np.concatenate(shards, axis=1)                  # [C, N, O]
    return np.ascontiguousarray(out).reshape(C, B, S, O), res


def kernel(x, w, scales):
    out, _ = run(x, w, scales, trace=False)
    return out


# revision 17
# speedup vs baseline: 1.1898x; 1.0032x over previous
"""Bass/Trainium2 kernel for per-chunk fake-quant + linear.

reference semantics (per chunk c):
    q  = clip(round(x/s_c), -128, 127) * s_c
    out[c] = q @ w[c].T          # [B,S,O]

Strategy (v2 - 16-bit I/O, ~32.5MB HBM traffic/core vs 68MB in v1):
  - Data-parallel over tokens: each of 8 cores gets T = B*S/8 = 8192 tokens
    (all 4 chunks), weights replicated.
  - x is staged host-side as f16 [C, D, T] (transposed so the contraction
    dim d sits on SBUF partitions). f16 keeps 11 mantissa bits: the induced
    quant-decision flips add ~0.2% rel error vs the 2e-2 tolerance.
  - Device quantization in two DVE passes (both 16-bit -> 4x perf mode):
      t16 = rne(x * 1/s)   as int16 (HW convert rounds RNE, range +-600)
      qi  = clip(t16, -128, 127) as f16 (exact small integers)
  - Weight-stationary matmuls: lhsT = ws16[c,dk,og] (128d x 128o), rhs =
    qi (128d x N tokens), PSUM tile [128o, 1024t] spanning 2 banks.
    ws16 = (s*w).T * 2^10 as f16 (kept normal), dequant 2^-10 folded into
    the PSUM->SBUF copy.
  - PSUM->SBUF copies at FD=1024 split ACT/DVE; output staged f16 and
    DMA'd out as f16 (host upcasts to f32).
  - DMA rings: x loads on sync HWDGE, weight + output stores on gpsimd
    SWDGE ring.
  - Post-passes: redundant back-to-back LDWEIGHTS with identical source
    APs are replaced by sync-preserving NOPs (weights already resident in
    the PE array); excess per-instruction sem waits hoisted onto NOPs.
"""

import numpy as np

import concourse.bass as bass
import concourse.tile as tile
import concourse.mybir as mybir
from concourse.bass_utils import run_bass_kernel_spmd


def _split_sync_waits(nc):
    """Hoist excess per-instruction sem waits onto preceding same-engine NOPs.

    This walrus build rejects instructions carrying >2 sync waits ("Too many
    sync wait commands", CoreV2/V3GenImpl setupSyncWait). A NOP on the same
    engine immediately before the instruction blocks the queue identically,
    so semantics are preserved.
    """
    count = 0
    for fn in nc.m.functions:
        for bb in fn.blocks:
            out = []
            for ins in bb.instructions:
                si = ins.sync_info
                waits = list(si.on_wait) if (si and si.on_wait) else []
                maxw = 1
                if len(waits) > maxw:
                    extra, keep = waits[:-maxw], waits[-maxw:]
                    ins.sync_info = mybir.SyncInfo(
                        on_wait=keep, on_update=list(si.on_update or [])
                    )
                    for j in range(0, len(extra), maxw):
                        count += 1
                        nop = mybir.InstNoOp(
                            name=f"ant-waitsplit-{count}", ins=[], outs=[]
                        )
                        nop.engine = ins.engine
                        nop.sync_info = mybir.SyncInfo(
                            on_wait=extra[j : j + maxw], on_update=[]
                        )
                        out.append(nop)
                out.append(ins)
            bb.instructions = out
    return count


def _dedupe_ldweights(nc):
    """Replace back-to-back InstLdweights with identical source APs by NOPs.

    The PE array keeps the stationary operand across matmuls; reloading the
    same weights between MMs only costs time. The NOP inherits the LDW's
    sync_info so queue blocking/semaphore semantics are unchanged.
    """
    pe = mybir.EngineType.PE
    count = 0
    for fn in nc.m.functions:
        for bb in fn.blocks:
            last_ldw_key = None
            out = []
            for ins in bb.instructions:
                ty = type(ins).__name__
                if getattr(ins, "engine", None) == pe:
                    if ty == "InstLdweights":
                        key = repr(ins.ins[0])
                        if key == last_ldw_key:
                            count += 1
                            nop = mybir.InstNoOp(
                                name=f"ant-ldwdedupe-{count}", ins=[], outs=[]
                            )
                            nop.engine = ins.engine
                            nop.sync_info = ins.sync_info
                            out.append(nop)
                            continue
                        last_ldw_key = key
                    elif ty in ("InstMatmult", "InstNoOp", "InstEventSemaphore"):
                        pass  # no effect on the PE weight registers
                    else:
                        last_ldw_key = None
                # non-PE instructions don't touch the PE weight registers
                out.append(ins)
            bb.instructions = out
    return count


C, B, S, D, O = 4, 8, 8192, 256, 256
NCORES = 8
N = B * S            # tokens per chunk (65536)
T = N // NCORES      # tokens per chunk per core (8192)

WS_SHIFT = 10           # weights pre-scaled by 2^10 to stay f16-normal
DEQUANT = float(2.0 ** -WS_SHIFT)

TG = 2048               # tokens per tile-group
N_TG = T // TG          # 4 tile-groups per chunk per core
MM_N = 512              # moving-operand width per matmul (PSUM one-bank limit)
PS_FD = 1024            # PSUM tile free dim (2 banks; copies read both)


def _build_program(scales):
    """Build the SPMD Bass program (same program on all cores).

    Inputs (per core): xt [C, D, T] f16, ws16 [C, D, O] f16.
    Output: out [C, 2, 128, T] f16  (o = og*128 + p, value = true_out).
    """
    f16 = mybir.dt.float16
    f32 = mybir.dt.float32
    i16 = mybir.dt.int16
    alu = mybir.AluOpType

    assert TG % PS_FD == 0 and PS_FD % MM_N == 0
    n_ps = TG // PS_FD         # PSUM tiles per og per tile-group
    n_mm = PS_FD // MM_N       # matmuls per PSUM tile per dk

    nc = bass.Bass()
    xt = nc.declare_dram_parameter("xt", [C, D, T], f16, isOutput=False)
    ws16 = nc.declare_dram_parameter("ws16", [C, D, O], f16, isOutput=False)
    out = nc.declare_dram_parameter("out", [C, 2, 128, T], f16, isOutput=True)

    with tile.TileContext(nc) as tc:
        with (
            tc.tile_pool(name="wpool", bufs=1) as wpool,
            tc.tile_pool(name="xpool", bufs=8) as xpool,
            tc.tile_pool(name="tpool", bufs=3) as tpool,
            tc.tile_pool(name="qpool", bufs=4) as qpool,
            tc.tile_pool(name="opool", bufs=5) as opool,
            tc.tile_pool(name="ppool", bufs=4, space=bass.MemorySpace.PSUM) as ppool,
        ):
            # Resident weights: wt[c, dk, og] = [128 d, 128 o] f16.
            w_tile = wpool.tile([128, 2 * C * O], f16, tag="w")
            nc.gpsimd.dma_start(
                out=w_tile[:].rearrange("p (g o) -> p g o", o=O),
                in_=ws16[:].rearrange("c (dk p) o -> p (c dk) o", p=128),
            )
            wt = {}
            for c in range(C):
                for dk in range(2):
                    for og in range(2):
                        base = (c * 2 + dk) * O + og * 128
                        wt[c, dk, og] = w_tile[:, base : base + 128]

            copy_rr = 0  # round-robin over the copy engines
            for c in range(C):
                inv_s = float(np.float32(1.0) / np.float32(scales[c]))
                for tg in range(N_TG):
                    # Load x tile: [p = d%128, (dk, t)]
                    x_tile = xpool.tile([128, 2 * TG], f16, tag="x")
                    src = xt[c].rearrange("(dk p) t -> p dk t", p=128)[
                        :, :, tg * TG : (tg + 1) * TG
                    ]
                    nc.sync.dma_start(
                        out=x_tile[:].rearrange("p (dk t) -> p dk t", dk=2),
                        in_=src,
                    )

                    # t16 = rne(x * inv_s) via the HW convert-on-write (RNE).
                    t16 = tpool.tile([128, 2 * TG], i16, tag="t16")
                    nc.vector.tensor_scalar(
                        t16[:], x_tile[:], inv_s, None, alu.mult
                    )
                    # qi = clip(t16, -128, 127) as f16 (exact integers)
                    qi = qpool.tile([128, 2 * TG], f16, tag="qi")
                    nc.vector.tensor_scalar(
                        qi[:], t16[:], -128.0, 127.0, alu.max, alu.min
                    )

                    # Matmuls, weight-stationary: ps [128 o, PS_FD t]
                    stage = opool.tile([128, 2 * TG], f16, tag="stage")
                    for og in range(2):
                        pss = [
                            ppool.tile([128, PS_FD], f32, name="ps", tag="ps")
                            for _ in range(n_ps)
                        ]
                        for dk in range(2):
                            for ips, ps in enumerate(pss):
                                for j in range(n_mm):
                                    t0 = ips * PS_FD + j * MM_N
                                    nc.tensor.matmul(
                                        ps[:, j * MM_N : (j + 1) * MM_N],
                                        wt[c, dk, og],
                                        qi[:, dk * TG + t0 : dk * TG + t0 + MM_N],
                                        start=(dk == 0),
                                        stop=(dk == 1),
                                    )
                        # PSUM -> SBUF staging with 2^-10 dequant folded in.
                        for ips, ps in enumerate(pss):
                            dst = stage[
                                :, og * TG + ips * PS_FD : og * TG + (ips + 1) * PS_FD
                            ]
                            # 3 ACT : 1 DVE balances measured engine busy
                            if copy_rr % 4 < 3:
                                nc.scalar.mul(dst, ps[:], DEQUANT)
                            else:
                                nc.vector.tensor_scalar(
                                    dst, ps[:], DEQUANT, None, alu.mult
                                )
                            copy_rr += 1

                    # Store both og halves in one 1MB SWDGE DMA:
                    # stage [p, (og t)] -> out[c, og, p, tg*TG:(tg+1)*TG]
                    if c < C - 1:
                        nc.gpsimd.dma_start(
                            out=out[c]
                            .rearrange("og p t -> p og t")[
                                :, :, tg * TG : (tg + 1) * TG
                            ],
                            in_=stage[:].rearrange("p (og t) -> p og t", og=2),
                        )
                    else:
                        # Last chunk: drain the tail over the (by now idle)
                        # HWDGE rings in parallel with Pool's earlier stores.
                        for og, eng in ((0, nc.sync), (1, nc.scalar)):
                            eng.dma_start(
                                out=out[c, og][:, tg * TG : (tg + 1) * TG],
                                in_=stage[:, og * TG : (og + 1) * TG],
                            )
    return nc


def _prep_inputs(x, w, scales, ncores=NCORES):
    x = np.ascontiguousarray(np.asarray(x, dtype=np.float32)).reshape(C, N, D)
    w = np.asarray(w, dtype=np.float32)
    s = np.asarray(scales, dtype=np.float32).reshape(C, 1, 1)

    ws = s * w                                            # [C, O, D] f32
    wsT = np.ascontiguousarray(ws.transpose(0, 2, 1))     # [C, D, O]
    ws16 = (wsT * np.float32(2.0**WS_SHIFT)).astype(np.float16)

    in_maps = []
    for i in range(ncores):
        xs = x[:, i * T : (i + 1) * T, :]                 # [C, T, D] view
        xtp = np.ascontiguousarray(
            xs.transpose(0, 2, 1).astype(np.float16)      # [C, D, T] f16
        )
        in_maps.append({"xt": xtp, "ws16": ws16})
    return in_maps


def run(x, w, scales, trace=False, **spmd_kwargs):
    """Compile + run on 8 cores. Returns (out, BassKernelResults)."""
    scales = np.asarray(scales, dtype=np.float32)
    nc = _build_program(scales)
    _dedupe_ldweights(nc)
    _split_sync_waits(nc)  # HW-only fixup (CoreSim chokes on raw-BIR NoOps)
    in_maps = _prep_inputs(x, w, scales)
    res = run_bass_kernel_spmd(
        nc, in_maps, core_ids=list(range(NCORES)), trace=trace, **spmd_kwargs
    )
    # Un-permute each shard: [C, 2, 128, T] (c, og, oj, t) -> [C, T, O]
    shards = [
        r["out"].reshape(C, O, T).transpose(0, 2, 1).astype(np.float32)
        for r in res.results
    ]
    out = np.concatenate(shards, axis=1)                  # [C, N, O]
    return np.ascontiguousarray(out).reshape(C, B, S, O), res


def kernel(x, w, scales):
    out, _ = run(x, w, scales, trace=False)
    return out
